# revision 23
# baseline (speedup 1.0000x reference)
"""Trainium2 Bass kernel for the Tacotron-style attention decoder.

Data-parallel over batch: 64 batches -> 8 NeuronCores x 8 batches.
All weights replicated (preprocessed/fused on host); the 500-step
recurrence runs locally per core with zero collectives.

Host-side preprocessing:
  - qk = h @ (Wq.T @ Wk) + bq @ Wk   (fuses query+key projections; the
    q.bk scalar score offset is softmax-invariant and dropped)
  - gates const = spk @ Wih_spk.T + b_ih + b_hh  (speaker embedding is
    step-invariant)
  - gate rows reordered (i,f,o,g); mel/stop projections fused
  - weights stored bf16 in the exact on-chip chunk layouts
  - enc stored twice in SBUF: h-major (scores pass) and s-major (ctx)

Dataflow per step (feature-major [128 partitions, batch-free] layouts):
  prenet form-B -> qk form-B -> scores (col-tiled, activations
  stationary, enc moving; batch b -> psum rows 32*(b%4)+16*(b//4)) ->
  exp/accum -> selector-gather to s-major -> ctx (col-tiled) ->
  selector-gather -> gates form-A (activations stationary, fused weight
  moving, col-tiled over gate quarters) -> selector-gather -> LSTM cell
  -> fused mel/stop projection.  Selector-gathers are matmuls against
  0/1 matrices that realize the batch-major -> feature-major transpose.
"""

import numpy as np
import ml_dtypes

B_FULL = 64
NB = 8          # batches per core
NCORES = 8
S = 1024
H = 512
NMELS = 80
T = 500
GATES = 2048

BF16 = ml_dtypes.bfloat16

_COMPILED = {}


# ---------------------------------------------------------------- host prep

def _prep_shared(W1, b1, W2, b2, Wq, bq, Wk, bk, W_ih, b_ih, W_hh, b_hh,
                 Wm, bm, Ws, bs):
    """Weight fusion + layout transforms shared by all cores."""
    f32 = np.float32
    W1, b1, W2, b2, Wq, bq, Wk, bk, W_ih, b_ih, W_hh, b_hh, Wm, bm, Ws, bs = [
        np.asarray(a, f32) for a in
        (W1, b1, W2, b2, Wq, bq, Wk, bk, W_ih, b_ih, W_hh, b_hh, Wm, bm, Ws, bs)]

    # fused query->key projection
    Wqk = Wq.T @ Wk                      # [512, 512] (in x out)
    bqk = bq @ Wk                        # [512]

    # gate reorder i,f,o,g
    ro = np.concatenate([np.arange(0, 512), np.arange(512, 1024),
                         np.arange(1536, 2048), np.arange(1024, 1536)])
    W_ihR = W_ih[ro]                     # [2048, 1536]
    W_hhR = W_hh[ro]                     # [2048, 512]
    b_ihR = b_ih[ro] + b_hh[ro]          # [2048]
    W_spkR = W_ihR[:, 1024:1536]         # [2048, 512]

    # combined gates weight, input chunks [p(512), h(512), ctx(512)] --
    # ctx last so attention-independent matmuls can run early.
    # form-A moving layout: wih[p, ic*2048 + o] = Win[o, ic*128 + p]
    Win = np.concatenate([W_ihR[:, 0:512], W_hhR, W_ihR[:, 512:1024]], axis=1)
    wih = (Win.T.reshape(12, 128, 2048).transpose(1, 0, 2)
           .reshape(128, 12 * 2048)).astype(BF16)

    def chunk44(WT):  # [512 in, 512 out] -> [128, (ic*4+oc)*128+m] (form-B)
        return (WT.reshape(4, 128, 4, 128).transpose(1, 0, 2, 3)
                .reshape(128, 4 * 4 * 128))

    w2T = chunk44(W2.T).astype(BF16)
    wqkT = chunk44(Wqk).astype(BF16)
    w1T = np.concatenate([W1.T, b1[None]], axis=0).astype(BF16)   # [81, 512]
    # fused mel+stop projection: [512, 81] -> chunks [128, 4*81]
    Wms = np.concatenate([Wm.T, Ws.T], axis=1)                    # [512, 81]
    wmsT = (Wms.reshape(4, 128, 81).transpose(1, 0, 2)
            .reshape(128, 4 * 81)).astype(BF16)
    bms = np.concatenate([bm, bs])[:, None].astype(f32)           # [81, 1]

    return dict(
        wih=wih, w2T=w2T, wqkT=wqkT, w1T=w1T, wmsT=wmsT, bms=bms,
        b2T=b2[None].astype(f32), bqkT=bqk[None].astype(f32),
        b2fm=b2.reshape(4, 128).T.copy().astype(f32),
        bqkfm=bqk.reshape(4, 128).T.copy().astype(f32),
        bmT=bm[None].astype(f32), bsT=bs[None].astype(f32),
        W_spkR=W_spkR, b_ihR=b_ihR,
    )


def _prep_core(enc_sh, spk_sh, shared):
    """Per-core tensors: enc layouts + gates constant."""
    f32 = np.float32
    enc_sh = np.asarray(enc_sh, f32)
    # h-major: [p, (hc*8+b)*1024 + s]
    encH = (enc_sh.transpose(2, 0, 1).reshape(4, 128, NB, S)
            .transpose(1, 0, 2, 3).reshape(128, 4 * NB * S)).astype(BF16)
    # s-major: [p, (b*8+sig)*512 + h]
    encS = (enc_sh.reshape(NB, 8, 128, H).transpose(2, 0, 1, 3)
            .reshape(128, NB * 8 * H)).astype(BF16)
    gc = spk_sh.astype(f32) @ shared["W_spkR"].T + shared["b_ihR"]  # [8, 2048]
    # gates layout [p, cc*32 + qd*8 + b], feature f = 512*qd + 128*cc + p
    gconst = (gc.T.reshape(4, 4, 128, NB).transpose(2, 1, 0, 3)
              .reshape(128, 128)).astype(f32)
    return dict(encH=encH, encS=encS, gconst=gconst)


class _StaticLoop:
    def __init__(self, rng):
        self.rng = rng

    def __enter__(self):
        return self.rng

    def __exit__(self, *a):
        return False


# ---------------------------------------------------------------- bass build

def _build(nsteps, reps=1, unrolled=False):
    import concourse.bass as bass
    import concourse.mybir as mybir
    import concourse.tile as tile
    from concourse import bacc
    from concourse.bass import ds

    f32 = mybir.dt.float32
    bf16 = mybir.dt.bfloat16
    AF = mybir.ActivationFunctionType
    MUL = mybir.AluOpType.mult
    ADD = mybir.AluOpType.add
    MAX = mybir.AluOpType.max

    nc = bacc.Bacc("TRN2", target_bir_lowering=False, debug=False)

    d_encH = nc.declare_dram_parameter("encH", [128, 4 * NB * S], bf16, isOutput=False)
    d_encS = nc.declare_dram_parameter("encS", [128, NB * 8 * H], bf16, isOutput=False)
    d_wih = nc.declare_dram_parameter("wih", [128, 12 * 2048], bf16, isOutput=False)
    d_w2T = nc.declare_dram_parameter("w2T", [128, 2048], bf16, isOutput=False)
    d_wqkT = nc.declare_dram_parameter("wqkT", [128, 2048], bf16, isOutput=False)
    d_w1T = nc.declare_dram_parameter("w1T", [81, 512], bf16, isOutput=False)
    d_wmsT = nc.declare_dram_parameter("wmsT", [128, 4 * 81], bf16, isOutput=False)
    d_bms = nc.declare_dram_parameter("bms", [81, 1], f32, isOutput=False)
    d_gconst = nc.declare_dram_parameter("gconst", [128, 128], f32, isOutput=False)
    d_b2fm = nc.declare_dram_parameter("b2fm", [128, 4], f32, isOutput=False)
    d_bqkfm = nc.declare_dram_parameter("bqkfm", [128, 4], f32, isOutput=False)
    d_ones = nc.declare_dram_parameter("ones_bf", [1, NB], bf16, isOutput=False)
    d_sel4 = nc.declare_dram_parameter("sel4", [128, 4], bf16, isOutput=False)
    d_sel8 = nc.declare_dram_parameter("sel8", [128, 32], bf16, isOutput=False)
    d_out = nc.declare_dram_parameter("melstop", [81, nsteps * NB], f32, isOutput=True)

    from contextlib import ExitStack, nullcontext
    with tile.TileContext(nc) as tc, ExitStack() as es:
        cpool = es.enter_context(tc.tile_pool(name="consts", bufs=1))
        wpool = es.enter_context(tc.tile_pool(name="work", bufs=2))
        pp_sc = es.enter_context(tc.tile_pool(name="ps_sc", bufs=1, space="PSUM"))
        pp_ctx = es.enter_context(tc.tile_pool(name="ps_ctx", bufs=2, space="PSUM"))
        pp_sm = es.enter_context(tc.tile_pool(name="ps_sm", bufs=2, space="PSUM"))

        s_encH = cpool.tile([128, 4 * NB * S], bf16)
        s_encS = cpool.tile([128, NB * 8 * H], bf16)
        s_wih = cpool.tile([128, 12 * 2048], bf16)
        s_w2T = cpool.tile([128, 2048], bf16)
        s_wqkT = cpool.tile([128, 2048], bf16)
        s_w1T = cpool.tile([81, 512], bf16)
        s_wmsT = cpool.tile([128, 4 * 81], bf16)
        s_bms = cpool.tile([81, 1], f32)
        s_gconst = cpool.tile([128, 128], f32)
        s_b2fm = cpool.tile([128, 4], f32)
        s_bqkfm = cpool.tile([128, 4], f32)
        s_sel4 = cpool.tile([128, 4], bf16)
        s_sel8 = cpool.tile([128, 32], bf16)
        s_melT = cpool.tile([81, NB], bf16)     # row 80 == 1.0 (bias row)
        s_hT4 = cpool.tile([128, 4 * 32], bf16)  # [hc, rep4, b]
        s_cT = cpool.tile([128, 4 * NB], f32)    # [cc, b]

        for dst, src in [(s_encH, d_encH), (s_encS, d_encS), (s_wih, d_wih),
                         (s_w2T, d_w2T), (s_wqkT, d_wqkT), (s_w1T, d_w1T),
                         (s_wmsT, d_wmsT), (s_bms, d_bms), (s_gconst, d_gconst),
                         (s_b2fm, d_b2fm), (s_bqkfm, d_bqkfm),
                         (s_sel4, d_sel4), (s_sel8, d_sel8)]:
            nc.sync.dma_start(out=dst[:], in_=src[:])

        nc.vector.memset(s_melT[0:80, :], 0.0)
        nc.sync.dma_start(out=s_melT[80:81, :], in_=d_ones[:])
        nc.vector.memset(s_hT4[:], 0.0)
        nc.vector.memset(s_cT[:], 0.0)

        def vap(tl, off, dims):
            return bass.AP(tensor=tl.tensor, offset=tl.offset + off,
                           ap=[tl.ap[0]] + dims)

        UNROLL = 5 if nsteps % 5 == 0 else 1
        rep_cm = nullcontext() if unrolled else tc.For_i(0, reps, 1)
        loop_cm = (_StaticLoop(range(0, nsteps * NB, NB * UNROLL)) if unrolled
                   else tc.For_i(0, nsteps * NB, NB * UNROLL))
        with rep_cm:
         with loop_cm as iv8s:
          for iv8 in (iv8s if unrolled else [iv8s]):
            for u in range(UNROLL):
                # ---- prenet layer 1: p1 = relu(mel @ W1.T + b1)  (form-B,
                # bias via the constant-1 row of melT)
                ps_p1 = pp_sm.tile([128, 4 * NB], f32, tag="sm")
                for oc in range(4):
                    nc.tensor.matmul(ps_p1[:, oc * NB:(oc + 1) * NB],
                                     s_w1T[:, oc * 128:(oc + 1) * 128],
                                     s_melT[:], start=True, stop=True)
                p1T = wpool.tile([128, 4 * NB], bf16, tag="p1T")
                nc.vector.tensor_relu(p1T[:], ps_p1[:])

                # ---- prenet layer 2: p2 = relu(p1 @ W2.T + b2)  (form-B)
                ps_p2 = pp_sm.tile([128, 4 * NB], f32, tag="sm")
                for oc in range(4):
                    o = ps_p2[:, oc * NB:(oc + 1) * NB]
                    for ic in range(4):
                        nc.tensor.matmul(
                            o, s_w2T[:, (ic * 4 + oc) * 128:(ic * 4 + oc + 1) * 128],
                            p1T[:, ic * NB:(ic + 1) * NB],
                            start=(ic == 0), stop=(ic == 3))
                # bias+relu, written 4x-replicated: p2T [oc, rep4, b]
                p2T = wpool.tile([128, 4 * 32], bf16, tag="p2T")
                for oc in range(4):
                    nc.vector.tensor_scalar(
                        vap(p2T, oc * 32, [[8, 4], [1, NB]]),
                        vap(ps_p2, oc * NB, [[0, 4], [1, NB]]),
                        s_b2fm[:, oc:oc + 1], 0.0, op0=ADD, op1=MAX)

                # ---- qk = h @ Wqk + bqk  (form-B)
                ps_qk = pp_sm.tile([128, 4 * NB], f32, tag="sm")
                for oc in range(4):
                    o = ps_qk[:, oc * NB:(oc + 1) * NB]
                    for ic in range(4):
                        nc.tensor.matmul(
                            o, s_wqkT[:, (ic * 4 + oc) * 128:(ic * 4 + oc + 1) * 128],
                            s_hT4[:, ic * 32:ic * 32 + NB],
                            start=(ic == 0), stop=(ic == 3))
                qkT = wpool.tile([128, 4 * NB], bf16, tag="qkT")
                for oc in range(4):
                    nc.vector.tensor_scalar(qkT[:, oc * NB:(oc + 1) * NB],
                                            ps_qk[:, oc * NB:(oc + 1) * NB],
                                            s_bqkfm[:, oc:oc + 1], None, op0=ADD)

                # ---- scores[b, s] = qk[b] . enc[b, s]
                # wave w = batches 4w..4w+4; batch j of a wave -> PE col
                # group j -> psum rows 32j..32j+32 (stationary replicated to
                # M=32); wave w occupies column half w of ps_sc [128, 2048].
                def rep32(col):
                    return bass.AP(tensor=col.tensor, offset=col.offset,
                                   ap=[col.ap[0], [0, 32]])

                ps_sc = pp_sc.tile([128, 2 * S], f32, tag="sc")
                for w in range(2):
                    for n2 in range(2):
                        for hc in range(4):
                            for j in range(4):
                                b = w * 4 + j
                                nc.tensor.matmul(
                                    ps_sc[32 * j:32 * j + 32,
                                          w * S + n2 * 512:w * S + (n2 + 1) * 512],
                                    rep32(qkT[:, hc * NB + b:hc * NB + b + 1]),
                                    s_encH[:, (hc * NB + b) * S + n2 * 512:
                                           (hc * NB + b) * S + (n2 + 1) * 512],
                                    start=(hc == 0), stop=(hc == 3),
                                    tile_position=(0, 32 * j),
                                    skip_group_check=True)
                attw = wpool.tile([128, 2 * S], bf16, tag="attw", bufs=1)
                ssum = wpool.tile([128, 2], f32, tag="ssum")
                rinv = wpool.tile([128, 2], f32, tag="rinv")
                for w in range(2):
                    nc.scalar.activation(attw[:, w * S:(w + 1) * S],
                                         ps_sc[:, w * S:(w + 1) * S], AF.Exp,
                                         accum_out=ssum[:, w:w + 1])
                nc.vector.reciprocal(rinv[:], ssum[:])

                # ---- gather attw -> s-major awT [128 s, sigma*8 + 4w + j]
                ps_awT = pp_sm.tile([128, 8 * NB], f32, tag="sm")
                for sg in range(8):
                    for w in range(2):
                        nc.tensor.matmul(
                            ps_awT[:, sg * NB + 4 * w:sg * NB + 4 * w + 4],
                            attw[:, w * S + sg * 128:w * S + (sg + 1) * 128],
                            s_sel4[:], start=True, stop=True)
                awT = wpool.tile([128, 8 * NB], bf16, tag="awT")
                nc.vector.tensor_copy(awT[:], ps_awT[:])

                # ---- ctx[b, h] = sum_s attw[b, s] enc[b, s, h]
                ctx_w = []
                for w in range(2):
                    ps_cx = pp_ctx.tile([128, H], f32, tag="ctx")
                    for sg in range(8):
                        for j in range(4):
                            b = w * 4 + j
                            nc.tensor.matmul(
                                ps_cx[32 * j:32 * j + 32, :],
                                rep32(awT[:, sg * NB + b:sg * NB + b + 1]),
                                s_encS[:, (b * 8 + sg) * H:(b * 8 + sg + 1) * H],
                                start=(sg == 0), stop=(sg == 7),
                                tile_position=(0, 32 * j),
                                skip_group_check=True)
                    ctx_sb = wpool.tile([128, H], bf16, tag=f"ctx_sb{w}", bufs=1)
                    nc.vector.tensor_scalar(ctx_sb[:], ps_cx[:],
                                            rinv[:, w:w + 1], None, op0=MUL)
                    ctx_w.append(ctx_sb)

                # ---- gather ctx -> ctxT4 [128, hc*32 + rep*8 + b]
                ps_cxT = pp_sm.tile([128, 4 * NB], f32, tag="sm")
                for hc in range(4):
                    for w in range(2):
                        nc.tensor.matmul(
                            ps_cxT[:, hc * NB + 4 * w:hc * NB + 4 * w + 4],
                            ctx_w[w][:, hc * 128:(hc + 1) * 128],
                            s_sel4[:], start=True, stop=True)
                ctxT = wpool.tile([128, 4 * 32], bf16, tag="ctxT")
                nc.vector.tensor_copy(
                    vap(ctxT, 0, [[32, 4], [8, 4], [1, NB]]),
                    vap(ps_cxT, 0, [[8, 4], [0, 4], [1, NB]]))

                # ---- gates: form-A, col-tiled over gate quarters (i,f,o,g)
                rhs_by_ic = [p2T] * 4 + [s_hT4] * 4 + [ctxT] * 4
                ps_gA = pp_ctx.tile([128, 512], f32, tag="ctx")
                for ic in range(12):
                    sl = rhs_by_ic[ic][:, (ic % 4) * 32:(ic % 4 + 1) * 32]
                    for qd in range(4):
                        nc.tensor.matmul(
                            ps_gA[32 * qd:32 * qd + 32, :], sl,
                            s_wih[:, ic * 2048 + qd * 512:
                                  ic * 2048 + (qd + 1) * 512],
                            start=(ic == 0), stop=(ic == 11),
                            tile_position=(0, 32 * qd),
                            skip_group_check=True)
                gA = wpool.tile([128, 512], bf16, tag="gA", bufs=1)
                nc.vector.tensor_copy(gA[:], ps_gA[:])
                # gather to [128, cc*32 + qd*8 + b]; f = 512*qd + 128*cc + p
                ps_g = pp_sm.tile([128, 128], f32, tag="sm")
                for cc in range(4):
                    nc.tensor.matmul(ps_g[:, cc * 32:(cc + 1) * 32],
                                     gA[:, cc * 128:(cc + 1) * 128],
                                     s_sel8[:], start=True, stop=True)
                g_sb = wpool.tile([128, 128], f32, tag="g_sb")
                nc.vector.scalar_tensor_tensor(g_sb[:], ps_g[:], 1.0,
                                               s_gconst[:], op0=MUL, op1=ADD)
                gact = wpool.tile([128, 128], f32, tag="gact")
                # sigmoid on i,f,o (cols 0..24 of each 32-block), tanh on g
                nc.scalar.activation(vap(gact, 0, [[32, 4], [1, 24]]),
                                     vap(g_sb, 0, [[32, 4], [1, 24]]),
                                     AF.Sigmoid)
                nc.scalar.activation(vap(gact, 24, [[32, 4], [1, NB]]),
                                     vap(g_sb, 24, [[32, 4], [1, NB]]),
                                     AF.Tanh)

                # ---- LSTM cell; views [cc4, b8]
                def gv(off):
                    return vap(gact, off, [[32, 4], [1, NB]])

                tmp1 = wpool.tile([128, 4 * NB], f32, tag="tmp1")
                tmp2 = wpool.tile([128, 4 * NB], f32, tag="tmp2")
                tanhc = wpool.tile([128, 4 * NB], f32, tag="tanhc")
                nc.vector.tensor_mul(tmp1[:], gv(0), gv(24))          # i*g
                nc.vector.tensor_mul(tmp2[:], gv(8), s_cT[:])         # f*c
                nc.vector.tensor_add(s_cT[:], tmp1[:], tmp2[:])
                nc.scalar.activation(tanhc[:], s_cT[:], AF.Tanh)
                nc.vector.tensor_mul(
                    vap(s_hT4, 0, [[32, 4], [8, 4], [1, NB]]),
                    vap(gact, 16, [[32, 4], [0, 4], [1, NB]]),
                    vap(tanhc, 0, [[8, 4], [0, 4], [1, NB]]))

                # ---- fused mel+stop projection (form-B, M=81)
                ps_ms = pp_sm.tile([128, NB], f32, tag="sm")
                for ic in range(4):
                    nc.tensor.matmul(ps_ms[0:81, :],
                                     s_wmsT[:, ic * 81:(ic + 1) * 81],
                                     s_hT4[:, ic * 32:ic * 32 + NB],
                                     start=(ic == 0), stop=(ic == 3))
                acc = wpool.tile([81, NB], f32, tag="acc")
                nc.vector.tensor_scalar(acc[:], ps_ms[0:81, :], s_bms[:],
                                        None, op0=ADD)
                nc.vector.tensor_scalar(s_melT[0:80, :], ps_ms[0:80, :],
                                        s_bms[0:80, :], None, op0=ADD)
                nc.sync.dma_start(out=d_out[:, ds(iv8 + u * NB, NB)],
                                  in_=acc[:])

    nc.compile()
    return nc


# ---------------------------------------------------------------- entry point

def _make_in_maps(inputs):
    shared = _prep_shared(**{k: inputs[k] for k in
                             ("W1", "b1", "W2", "b2", "Wq", "bq", "Wk", "bk",
                              "W_ih", "b_ih", "W_hh", "b_hh", "Wm", "bm", "Ws", "bs")})
    enc = np.asarray(inputs["encoder_output"], np.float32)
    spk = np.asarray(inputs["speaker_embedding"], np.float32)
    sel4 = np.zeros((128, 4), BF16)
    for j in range(4):
        sel4[32 * j, j] = 1.0
    sel8 = np.zeros((128, 32), BF16)
    for q in range(4):
        for b in range(NB):
            sel8[32 * q + b, q * NB + b] = 1.0
    in_maps = []
    for c in range(NCORES):
        core = _prep_core(enc[c * NB:(c + 1) * NB], spk[c * NB:(c + 1) * NB],
                          shared)
        m = {k: shared[k] for k in ("wih", "w2T", "wqkT", "w1T", "wmsT",
                                    "bms", "b2fm", "bqkfm")}
        m.update(encH=core["encH"], encS=core["encS"], gconst=core["gconst"],
                 ones_bf=np.ones((1, NB), BF16), sel4=sel4, sel8=sel8)
        in_maps.append(m)
    return in_maps


def kernel(encoder_output, speaker_embedding, max_steps,
           W1, b1, W2, b2, Wq, bq, Wk, bk,
           W_ih, b_ih, W_hh, b_hh, Wm, bm, Ws, bs):
    from concourse.bass_utils import run_bass_kernel_spmd

    nsteps = int(max_steps)
    in_maps = _make_in_maps(dict(
        encoder_output=encoder_output, speaker_embedding=speaker_embedding,
        W1=W1, b1=b1, W2=W2, b2=b2, Wq=Wq, bq=bq, Wk=Wk, bk=bk,
        W_ih=W_ih, b_ih=b_ih, W_hh=W_hh, b_hh=b_hh, Wm=Wm, bm=bm, Ws=Ws, bs=bs))

    if nsteps not in _COMPILED:
        _COMPILED[nsteps] = _build(nsteps)
    nc = _COMPILED[nsteps]

    res = run_bass_kernel_spmd(nc, in_maps, core_ids=list(range(NCORES)))

    mels = np.empty((B_FULL, nsteps, NMELS), np.float32)
    stops = np.empty((B_FULL, nsteps), np.float32)
    for c in range(NCORES):
        ms = res.results[c]["melstop"]                     # [81, nsteps*8]
        mels[c * NB:(c + 1) * NB] = (ms[0:80].reshape(80, nsteps, NB)
                                     .transpose(2, 1, 0))
        stops[c * NB:(c + 1) * NB] = ms[80].reshape(nsteps, NB).T
    return mels, stops


# ---------------------------------------------------------------- numpy sim

def sim_kernel(encoder_output, speaker_embedding, max_steps,
               W1, b1, W2, b2, Wq, bq, Wk, bk,
               W_ih, b_ih, W_hh, b_hh, Wm, bm, Ws, bs, use_bf16=True):
    """Numpy re-implementation of the exact kernel math (for validation)."""
    def q(x):
        return x.astype(BF16).astype(np.float32) if use_bf16 else x

    nsteps = int(max_steps)
    shared = _prep_shared(W1, b1, W2, b2, Wq, bq, Wk, bk,
                          W_ih, b_ih, W_hh, b_hh, Wm, bm, Ws, bs)
    enc = q(np.asarray(encoder_output, np.float32))
    spk = np.asarray(speaker_embedding, np.float32)

    def unchunk44(wt):
        return (wt.astype(np.float32).reshape(128, 4, 4, 128)
                .transpose(1, 0, 2, 3).reshape(512, 512))

    Wqk = unchunk44(shared["wqkT"]); bqk = shared["bqkT"][0]
    W2T = unchunk44(shared["w2T"]); b2v = shared["b2T"][0]
    W1T = shared["w1T"].astype(np.float32)                  # [81, 512]
    Win = (shared["wih"].astype(np.float32).reshape(128, 12, 2048)
           .transpose(1, 0, 2).reshape(1536, 2048))
    Wms = (shared["wmsT"].astype(np.float32).reshape(128, 4, 81)
           .transpose(1, 0, 2).reshape(512, 81))
    bmsv = shared["bms"][:, 0]

    B = enc.shape[0]
    gc = spk @ shared["W_spkR"].T + shared["b_ihR"]         # [B, 2048]
    mel = np.zeros((B, NMELS), np.float32)
    h = np.zeros((B, H), np.float32)
    c = np.zeros((B, H), np.float32)
    mels = np.zeros((B, nsteps, NMELS), np.float32)
    stops = np.zeros((B, nsteps), np.float32)
    sig = lambda v: 1.0 / (1.0 + np.exp(-v))

    for t in range(nsteps):
        p1 = np.maximum(q(mel) @ W1T[:80] + W1T[80], 0)
        p2 = np.maximum(q(p1) @ W2T + b2v, 0)
        qk = q(q(h) @ Wqk + bqk)
        scores = np.einsum('bh,bsh->bs', qk, enc)
        e = np.exp(scores)
        attw = q(e) / e.sum(-1, keepdims=True)
        ctx = np.einsum('bs,bsh->bh', attw, enc)
        x = np.concatenate([q(p2), q(h), q(ctx)], axis=-1)
        gates = q(x @ Win) + gc                              # i,f,o,g
        i, f, o, g = np.split(gates, 4, axis=-1)
        c = sig(f) * c + sig(i) * np.tanh(g)
        h = sig(o) * np.tanh(c)
        ms = q(h) @ Wms + bmsv                               # [B, 81]
        mel = ms[:, 0:80]
        stops[:, t] = ms[:, 80]
        mels[:, t] = mel
    return mels, stops
